# revision 10
# baseline (speedup 1.0000x reference)
"""Bahdanau-attention kernel for Trainium2 (8 NeuronCores, batch-sharded).

The reference computes

    score  = tanh(features @ W1 + b1 + hidden @ W2 + b2) @ V + bv   # [B, 1, 1]
    attn   = softmax(score, axis=1)                                 # axis of size 1!
    context = sum(attn * features[:, None, :], axis=1)              # [B, D]

The softmax is taken over an axis of size 1, so `attn == 1.0` exactly and
`context == features` bitwise — the two matmuls and the tanh are dead code.
The roofline for this module is therefore pure memory traffic: read the
features shard and write it back as `context`, plus a tiny ones tensor.

Each core owns B/8 = 2048 rows: DRAM->DRAM DMA copy of its [2048, 2048] f32
features shard into the context output, and a memset(1.0) tile stored to the
attention-weights output.
"""

import os

import numpy as np

import concourse.bass as bass
import concourse.tile as tile
from concourse import mybir
from concourse.bass_utils import run_bass_kernel_spmd

N_CORES = 8
B, D, H = 16384, 2048, 1024
ROWS = B // N_CORES  # 2048 rows per core
N_SYNC_CHUNKS = 4  # copy chunks issued from the sync (SP) HWDGE ring
N_SCALAR_CHUNKS = 4  # copy chunks issued from the scalar (ACT) HWDGE ring

LAST_EXEC_TIME_NS = None
LAST_RESULTS = None


def _build_nc():
    nc = bass.Bass(trn_type="TRN2")
    x = nc.dram_tensor("x", [ROWS, D], mybir.dt.float32, kind="ExternalInput")
    ctx_out = nc.dram_tensor("ctx", [ROWS, D], mybir.dt.float32, kind="ExternalOutput")
    attn_out = nc.dram_tensor(
        "attn", [128, ROWS // 128], mybir.dt.float32, kind="ExternalOutput"
    )

    total = N_SYNC_CHUNKS + N_SCALAR_CHUNKS
    rows_per = ROWS // total
    chunks = [slice(i * rows_per, (i + 1) * rows_per) for i in range(total)]
    sync_chunks = chunks[:N_SYNC_CHUNKS]
    scalar_chunks = chunks[N_SYNC_CHUNKS:]

    with (
        nc.sbuf_tensor([128, ROWS // 128], mybir.dt.float32) as ones,
        nc.semaphore("vsem") as vsem,
        nc.semaphore("attn_sem") as attn_sem,
        nc.semaphore("dma_sem") as dma_sem,
        nc.Block() as block,
    ):
        # The ones/attn path lives entirely on gpsimd (SWDGE, own semaphore),
        # keeping both HWDGE rings free to stream the big copy from t=0.
        @block.gpsimd
        def _(gpsimd):
            gpsimd.memset(ones[:], 1.0).then_inc(vsem, 1)
            gpsimd.wait_ge(vsem, 1)
            gpsimd.dma_start(out=attn_out[:, :], in_=ones[:]).then_inc(attn_sem, 16)

        @block.scalar
        def _(scalar):
            for sl in scalar_chunks:
                scalar.dma_start(out=ctx_out[sl, :], in_=x[sl, :]).then_inc(
                    dma_sem, 16
                )

        @block.sync
        def _(sync):
            for sl in sync_chunks:
                sync.dma_start(out=ctx_out[sl, :], in_=x[sl, :]).then_inc(dma_sem, 16)
            sync.wait_ge(dma_sem, 16 * total)
            sync.wait_ge(attn_sem, 16)

    return nc


def kernel(features, hidden, W1, b1, W2, b2, V, bv):
    global LAST_EXEC_TIME_NS, LAST_RESULTS
    features = np.ascontiguousarray(np.asarray(features, dtype=np.float32))
    assert features.shape == (B, D)

    nc = _build_nc()
    in_maps = [{"x": features[i * ROWS : (i + 1) * ROWS]} for i in range(N_CORES)]
    trace = bool(os.environ.get("KERNEL_TRACE"))
    res = run_bass_kernel_spmd(
        nc, in_maps, core_ids=list(range(N_CORES)), trace=trace
    )
    LAST_EXEC_TIME_NS = res.exec_time_ns
    LAST_RESULTS = res

    context = np.concatenate([r["ctx"] for r in res.results], axis=0)
    attn = np.concatenate(
        [r["attn"].reshape(-1) for r in res.results]
    ).reshape(B, 1, 1)
    return context, attn
